# revision 2
# baseline (speedup 1.0000x reference)
"""MoE router gate (DeepSeek-V3 style) on 8 Trainium2 NeuronCores.

v6: v5 + host pre-tiled DRAM layout. Every SBUF tile's bytes are stored
contiguously per partition in DRAM (4-8KB DMA lines, no gather pattern),
which measures 370 GB/s/core vs 276 GB/s for the d-major + rearrange
pattern. DMA is the roofline for this kernel (58.7MB x + 7.4MB w per
core), so line size directly sets the floor.

GEMM (layout B, w stationary, x streaming; per 56 K-chunks x 2 expert
halves, h-interleaved fp16 / fp8-DoubleRow so LDWEIGHTS hides):
  logits.T * 2^28 = fp16(x*2^12) @ fp16(w*2^16)
                  + [xl*2^16 ; x*2^5]_fp8 @DR [w*2^12 ; wl*2^23]_fp8
PSUM [expert, token] result is PE-transposed back to [token, expert];
sigmoid applies 2^-28. Logit error ~2^-15 -> ~11/16384 flipped tokens.

Progressive token blocks [128,128,256,512,512,256,256] start the routing
pipeline early and shorten the serial tail.

Sharding: data-parallel over tokens (2048/core); w+bias replicated.
"""

import sys
import threading

sys.path.insert(0, "/opt/trn_rl_repo")

import numpy as np
import ml_dtypes

import concourse.bass as bass
import concourse.bacc as bacc
import concourse.mybir as mybir
import concourse.tile as tile
from concourse.masks import make_identity
from concourse.bass_utils import run_bass_kernel_spmd

N_TOK = 16384
D = 7168
E = 256
EH = E // 2
N_CORES = 8
NSH = N_TOK // N_CORES          # tokens per core
TOK_TILE = 128
TOK_BLOCK = 512                 # max block (SBUF tile size)
BLOCKS = [128, 512, 512, 512, 384]      # small first block -> early routing
BSIZE_TAG = {0: 's0'}                    # exact-size tags; block 4 reuses 512 tiles partially
BSTART = [sum(BLOCKS[:i]) for i in range(len(BLOCKS))]
assert sum(BLOCKS) == NSH
KC = 128
N_KC = D // KC                  # 56
N_GROUPS = 8
GSIZE = E // N_GROUPS           # 32
TOPK_GROUPS = 4
TOPK = 8
ROUTE_SCALE = 2.5
NEG_BIG = 1.0e30
SCALE_S = 2.0 ** 28

# chunk groups (stagger weight/x loads so chunk-0 matmuls start early)
GS = [2, 6] + [8] * 6
GOFF = [sum(GS[:i]) for i in range(len(GS))]
NG = len(GS)
C2G = []
for _gi, _n in enumerate(GS):
    C2G += [(_gi, _c) for _c in range(_n)]

# flat-tensor offsets (elements) for the pre-tiled layouts
XH_OFF, XC_OFF, WH_OFF, WC_OFF = {}, {}, {}, {}
_o = 0
for _b in range(len(BLOCKS)):
    for _g in range(NG):
        XH_OFF[(_b, _g)] = _o
        _o += 128 * GS[_g] * BLOCKS[_b]
XH_TOT = _o
_o = 0
for _b in range(len(BLOCKS)):
    for _g in range(NG):
        XC_OFF[(_b, _g)] = _o
        _o += 128 * GS[_g] * BLOCKS[_b]
XC_TOT = _o
_o = 0
for _g in range(NG):
    WH_OFF[_g] = _o
    _o += 128 * GS[_g] * E
WH_TOT = _o
_o = 0
for _g in range(NG):
    WC_OFF[_g] = _o
    _o += 128 * GS[_g] * 2 * E
WC_TOT = _o

_cached = {}


def _build_nc():
    """Per-core bass program. SPMD: same program, per-core input maps."""
    fp16 = mybir.dt.float16
    fp8 = mybir.dt.float8e4
    f32 = mybir.dt.float32
    u32 = mybir.dt.uint32
    u16 = mybir.dt.uint16
    DR = mybir.MatmulPerfMode.DoubleRow

    nc = bacc.Bacc(trn_type="TRN2", target_bir_lowering=False)

    xh_d = nc.dram_tensor("xh", [XH_TOT], fp16, kind="ExternalInput")
    xc_d = nc.dram_tensor("xc", [XC_TOT], fp8, kind="ExternalInput")
    wh_d = nc.dram_tensor("wh", [WH_TOT], fp16, kind="ExternalInput")
    wc_d = nc.dram_tensor("wc", [WC_TOT], fp8, kind="ExternalInput")
    bias_d = nc.dram_tensor("bias", [128, E], f32, kind="ExternalInput")
    wts_d = nc.dram_tensor("wts", [NSH, TOPK], f32, kind="ExternalOutput")
    idx_d = nc.dram_tensor("idx", [NSH, TOPK], mybir.dt.int32, kind="ExternalOutput")

    with tile.TileContext(nc) as tc:
        with (
            tc.tile_pool(name="wpool", bufs=1) as wpool,
            tc.tile_pool(name="xpool", bufs=1) as xpool,
            tc.tile_pool(name="spool", bufs=2) as spool,
            tc.tile_pool(name="cpool", bufs=1) as cpool,
            tc.tile_pool(name="tiny", bufs=1) as tiny,
            tc.tile_pool(name="accp", bufs=2, space="PSUM") as accp,
            tc.tile_pool(name="ptp", bufs=4, space="PSUM") as ptp,
        ):
            wh_g, wc_g = [], []
            xh_gb = [None] * NG
            xc_gb = [None] * NG

            def load_x_group(g, b):
                T = BLOCKS[b]
                sz = BSIZE_TAG.get(b)
                TT = T if sz else TOK_BLOCK
                tag = sz or ""
                xhg = xpool.tile([128, GS[g], TT], fp16, tag=f"xh{tag}{g}", bufs=1)
                off = XH_OFF[(b, g)]
                nc.sync.dma_start(
                    xhg[:, :, 0:T],
                    xh_d[off : off + 128 * GS[g] * T].rearrange(
                        "(p c n) -> p c n", p=128, c=GS[g]
                    ),
                )
                xh_gb[g] = xhg
                xcg = xpool.tile([128, 2, GS[g], TT], fp8, tag=f"xc{tag}{g}", bufs=1)
                off = XC_OFF[(b, g)]
                nc.gpsimd.dma_start(
                    xcg[:, 0, :, 0:T],
                    xc_d[off : off + 128 * GS[g] * T].rearrange(
                        "(p c n) -> p c n", p=128, c=GS[g]
                    ),
                )
                # derive the second fp8 operand on device: q8(x*2^5) =
                # cast(fp16(x)*2^12 * 2^-7); saves a 14.7MB/core DMA stream
                nc.scalar.activation(
                    xcg[:, 1, :, 0:T], xhg[:, :, 0:T],
                    mybir.ActivationFunctionType.Copy, scale=2.0 ** -7,
                )
                xc_gb[g] = xcg

            for g in range(NG):
                whg = wpool.tile([128, GS[g], E], fp16, tag=f"wh{g}", bufs=1)
                off = WH_OFF[g]
                nc.scalar.dma_start(
                    whg[:, :, :],
                    wh_d[off : off + 128 * GS[g] * E].rearrange(
                        "(p c e) -> p c e", p=128, c=GS[g]
                    ),
                )
                wh_g.append(whg)
                wcg = wpool.tile([128, GS[g], 2, E], fp8, tag=f"wc{g}", bufs=1)
                off = WC_OFF[g]
                nc.scalar.dma_start(
                    wcg[:, :, :, :],
                    wc_d[off : off + 128 * GS[g] * 2 * E].rearrange(
                        "(p c k e) -> p c k e", p=128, c=GS[g], k=2
                    ),
                )
                wc_g.append(wcg)
                load_x_group(g, 0)
            bias_sb = wpool.tile([128, E], f32, tag="bias")
            nc.scalar.dma_start(bias_sb[:, :], bias_d[:, :])
            ident = wpool.tile([128, 128], f32, tag="ident")
            make_identity(nc, ident)

            for b in range(len(BLOCKS)):
                T = BLOCKS[b]
                if b > 0:
                    for g in range(NG):
                        load_x_group(g, b)

                # acc[:, h, :T] = (logits.T * 2^28) for expert half h;
                # h-interleaved fp16/DR order keeps LDWEIGHTS hidden
                acc = accp.tile([128, 2, TOK_BLOCK], f32, tag="acc")
                for g in range(NG):
                    for ci in range(GS[g]):
                        for h in range(2):
                            nc.tensor.matmul(
                                acc[:, h, 0:T],
                                wh_g[g][:, ci, h * EH : (h + 1) * EH],
                                xh_gb[g][:, ci, 0:T],
                                start=(g == 0 and ci == 0),
                                stop=False,
                            )
                    for ci in range(GS[g]):
                        for h in range(2):
                            nc.tensor.matmul(
                                acc[:, h, 0:T],
                                wc_g[g][:, ci, :, h * EH : (h + 1) * EH],
                                xc_gb[g][:, :, ci, 0:T],
                                start=False,
                                stop=(g == NG - 1 and ci == GS[g] - 1),
                                perf_mode=DR,
                            )

                # PSUM -> SBUF so PE can transpose (PE reads SBUF only)
                cp = []
                for h in range(2):
                    cph = cpool.tile([128, TOK_BLOCK], f32, tag=f"cp{h}")
                    nc.scalar.activation(
                        cph[:, 0:T], acc[:, h, 0:T],
                        mybir.ActivationFunctionType.Copy,
                    )
                    cp.append(cph)

                nsub = T // TOK_TILE
                wout_blk = cpool.tile([128, 4, TOPK], f32, tag="wout_blk")
                idx_blk = cpool.tile([128, 4, TOPK], u32, tag="idx_blk")
                for t4 in range(nsub):
                    ts = BSTART[b] + t4 * TOK_TILE
                    psT = ptp.tile([128, 2 * E], f32, tag="psT")
                    for h in range(2):
                        nc.tensor.transpose(
                            psT[:, h * EH : (h + 1) * EH],
                            cp[h][:, t4 * TOK_TILE : (t4 + 1) * TOK_TILE],
                            ident[:, :],
                        )
                    ps = psT[:, 0:E]

                    # scores = sigmoid(logits * 2^-28); s = scores + bias
                    scores = spool.tile([128, E], f32, tag="scores")
                    nc.scalar.activation(
                        scores[:, :], ps[:, :],
                        mybir.ActivationFunctionType.Sigmoid,
                        scale=1.0 / SCALE_S,
                    )
                    s = spool.tile([128, E], f32, tag="s")
                    nc.vector.tensor_add(s[:, :], scores[:, :], bias_sb[:, :])

                    gtop = tiny.tile([128, N_GROUPS, 8], f32, tag="gtop")
                    for g in range(N_GROUPS):
                        nc.vector.max(
                            gtop[:, g, :], s[:, g * GSIZE : (g + 1) * GSIZE]
                        )
                    gs = tiny.tile([128, N_GROUPS], f32, tag="gs")
                    nc.vector.tensor_add(gs[:, :], gtop[:, :, 0], gtop[:, :, 1])

                    gsort = tiny.tile([128, 8], f32, tag="gsort")
                    nc.vector.max(gsort[:, :], gs[:, :])
                    keep = tiny.tile([128, N_GROUPS], f32, tag="keep")
                    nc.vector.tensor_scalar(
                        keep[:, :], gs[:, :], gsort[:, 3:4], None,
                        op0=mybir.AluOpType.is_ge,
                    )
                    amask = tiny.tile([128, N_GROUPS], f32, tag="amask")
                    nc.vector.tensor_scalar(
                        amask[:, :], keep[:, :], 1.0, NEG_BIG,
                        op0=mybir.AluOpType.subtract, op1=mybir.AluOpType.mult,
                    )

                    # in-place: s += amask (broadcast per group) -> masked s
                    nc.vector.tensor_tensor(
                        s[:, :].rearrange("p (g e) -> p g e", g=N_GROUPS),
                        s[:, :].rearrange("p (g e) -> p g e", g=N_GROUPS),
                        amask[:, :, None].to_broadcast([128, N_GROUPS, GSIZE]),
                        op=mybir.AluOpType.add,
                    )

                    smask2 = s[:, :]
                    top8v = tiny.tile([128, TOPK], f32, tag="top8v")
                    nc.vector.max(top8v[:, :], smask2)
                    top8i = tiny.tile([128, TOPK], u32, tag="top8i")
                    nc.vector.max_index(top8i[:, :], top8v[:, :], smask2)

                    wsel = tiny.tile([128, TOPK], f32, tag="wsel")
                    scratch = spool.tile([128, E], f32, tag="scratch", bufs=1)
                    for j in range(TOPK):
                        nc.vector.scalar_tensor_tensor(
                            scratch[:, :], smask2, top8v[:, j : j + 1],
                            scores[:, :],
                            op0=mybir.AluOpType.is_equal,
                            op1=mybir.AluOpType.mult,
                            accum_out=wsel[:, j : j + 1],
                        )

                    ssum = tiny.tile([128, 1], f32, tag="ssum")
                    nc.vector.reduce_sum(
                        ssum[:, :], wsel[:, :], axis=mybir.AxisListType.X
                    )
                    rec = tiny.tile([128, 1], f32, tag="rec")
                    nc.vector.reciprocal(rec[:, :], ssum[:, :])
                    wout = tiny.tile([128, TOPK], f32, tag="wout")
                    nc.vector.tensor_scalar(
                        wout[:, :], wsel[:, :], rec[:, 0:1], ROUTE_SCALE,
                        op0=mybir.AluOpType.mult, op1=mybir.AluOpType.mult,
                    )

                    nc.scalar.dma_start(wts_d[ts : ts + TOK_TILE, :], wout[:, :])
                    nc.scalar.dma_start(
                        idx_d[ts : ts + TOK_TILE, :],
                        top8i[:, :].bitcast(mybir.dt.int32),
                    )
    nc.finalize()
    return nc


def _tile_pg(arr_cd, b, g):
    """[GS[g]*128, T]-sliced chunk rows -> [128, GS[g], T] partition-major."""
    r0, r1 = GOFF[g] * 128, (GOFF[g] + GS[g]) * 128
    bs, T = BSTART[b], BLOCKS[b]
    sub = arr_cd[r0:r1, bs : bs + T]
    return sub.reshape(GS[g], 128, T).transpose(1, 0, 2)


def _host_prep(x, weight, bias):
    """Split to fp16 hi + fp8 correction streams, pre-tiled per-core."""
    x = np.asarray(x, dtype=np.float32)
    weight = np.asarray(weight, dtype=np.float32)
    bias = np.asarray(bias, dtype=np.float32)
    f8 = ml_dtypes.float8_e4m3

    wT = np.ascontiguousarray(weight.T)             # [D, E]
    wh16 = (wT * 2.0 ** 16).astype(np.float16)      # fp16(w)*2^16, exact shift
    wl = wT - wh16.astype(np.float32) * 2.0 ** -16
    wc8 = np.clip(wT * 2.0 ** 12, -240, 240).astype(f8)
    wl8 = np.clip(wl * 2.0 ** 23, -240, 240).astype(f8)

    wh_flat = np.empty(WH_TOT, dtype=np.float16)
    wc_flat = np.empty(WC_TOT, dtype=f8)
    for g in range(NG):
        r0, r1 = GOFF[g] * 128, (GOFF[g] + GS[g]) * 128
        wh_flat[WH_OFF[g] : WH_OFF[g] + 128 * GS[g] * E] = (
            wh16[r0:r1].reshape(GS[g], 128, E).transpose(1, 0, 2).ravel()
        )
        pair = np.stack(
            [wc8[r0:r1].reshape(GS[g], 128, E), wl8[r0:r1].reshape(GS[g], 128, E)],
            axis=2,
        )  # [GS, 128, 2, E]
        wc_flat[WC_OFF[g] : WC_OFF[g] + 128 * GS[g] * 2 * E] = pair.transpose(
            1, 0, 2, 3
        ).ravel()
    bias_rep = np.ascontiguousarray(np.broadcast_to(bias[None, :], (128, E)))

    in_maps = [None] * N_CORES

    def prep_core(c):
        xsT = x[c * NSH : (c + 1) * NSH, :].T       # [D, NSH] view
        a = xsT.astype(np.float16)
        xh16 = a * np.float16(2.0 ** 12)
        xl8 = ((xsT - a.astype(np.float32)) * 2.0 ** 16).astype(f8)
        xh_flat = np.empty(XH_TOT, dtype=np.float16)
        xc_flat = np.empty(XC_TOT, dtype=f8)
        for b in range(len(BLOCKS)):
            T = BLOCKS[b]
            for g in range(NG):
                n = 128 * GS[g] * T
                xh_flat[XH_OFF[(b, g)] : XH_OFF[(b, g)] + n] = _tile_pg(
                    xh16, b, g
                ).ravel()
                xc_flat[XC_OFF[(b, g)] : XC_OFF[(b, g)] + n] = _tile_pg(
                    xl8, b, g
                ).ravel()
        in_maps[c] = {
            "xh": xh_flat,
            "xc": xc_flat,
            "wh": wh_flat,
            "wc": wc_flat,
            "bias": bias_rep,
        }

    threads = [threading.Thread(target=prep_core, args=(c,)) for c in range(N_CORES)]
    for th in threads:
        th.start()
    for th in threads:
        th.join()
    return in_maps


def kernel(x, weight, bias, _trace=False):
    if "nc" not in _cached:
        _cached["nc"] = _build_nc()
    nc = _cached["nc"]
    in_maps = _host_prep(x, weight, bias)
    res = run_bass_kernel_spmd(
        nc, in_maps, core_ids=list(range(N_CORES)), trace=_trace
    )
    _cached["last_result"] = res
    wts = np.concatenate([r["wts"] for r in res.results], axis=0)
    idx = np.concatenate([r["idx"] for r in res.results], axis=0)
    return wts, idx
